# revision 8
# baseline (speedup 1.0000x reference)
"""Trainium2 Bass kernel for a 2-layer LSTM encoder (B=64, T=128, D=256, H=1024)
with embedding lookup and inference BatchNorm.

Sharding: model-parallel over the hidden dimension H (each of 8 cores owns a
128-wide H chunk = 512 of the 4096 gate columns). The recurrence is sequential
in T; after each step every core broadcasts its h chunk to all cores via an
AllGather so the next step's h @ U matmul has the full hidden state.

Matmul orientation: out z[t] = [B=64, 512_own_gates] with the small h^T / x^T
chunks as the stationary operand and the (SBUF-resident) weight chunks as the
moving operand at N=512 — float32r streams at full rate for N >= 256.
"""

import sys

import numpy as np

for _p in ("/opt/trn_rl_repo", "/root/.axon_site/_ro/trn_rl_repo"):
    if _p not in sys.path:
        sys.path.append(_p)

B = 64
T = 128
V = 32000
D = 256
H = 1024
NCORES = 8
HC = H // NCORES        # 128 hidden dims owned per core
G = 4 * HC              # 512 gate columns owned per core
KH = H // 128           # 8 K-chunks for H contraction
KD = D // 128           # 2 K-chunks for D contraction
BN_EPS = 1e-3

_COMPILED = {}


def _gate_cols(k):
    """Columns of the 4H gate dim owned by core k (Keras order i,f,g,o)."""
    return np.concatenate(
        [np.arange(g * H + k * HC, g * H + (k + 1) * HC) for g in range(4)]
    )


def build_program(t_steps=T):
    import concourse.bass as bass
    import concourse.bacc as bacc
    import concourse.mybir as mybir
    import concourse.tile as tile
    from concourse._compat import axon_active

    f32 = mybir.dt.float32
    f32r = mybir.dt.float32r
    i32 = mybir.dt.int32
    AF = mybir.ActivationFunctionType

    nc = bacc.Bacc(
        "TRN2",
        target_bir_lowering=False,
        debug=not axon_active(),
        enable_asserts=True,
        num_devices=NCORES,
    )

    # ---- DRAM I/O (per-core shards supplied via in_maps) ----
    emb_d = nc.dram_tensor("emb", [V, D], f32, kind="ExternalInput")
    tok_d = nc.dram_tensor("tok", [B, t_steps], i32, kind="ExternalInput")
    u0_d = nc.dram_tensor("u0", [KH, 128, G], f32, kind="ExternalInput")
    u1_d = nc.dram_tensor("u1", [KH, 128, G], f32, kind="ExternalInput")
    w1_d = nc.dram_tensor("w1", [KH, 128, G], f32, kind="ExternalInput")
    w0_d = nc.dram_tensor("w0", [KD, 128, G], f32, kind="ExternalInput")
    b0_d = nc.dram_tensor("b0", [1, G], f32, kind="ExternalInput")
    b1_d = nc.dram_tensor("b1", [1, G], f32, kind="ExternalInput")
    h0t_d = nc.dram_tensor("h0t", [H, B], f32, kind="ExternalInput")
    c0_d = nc.dram_tensor("c0sh", [B, HC], f32, kind="ExternalInput")
    sb_d = nc.dram_tensor("sb", [B, HC], f32, kind="ExternalInput")
    tb_d = nc.dram_tensor("tb", [B, HC], f32, kind="ExternalInput")
    id_d = nc.dram_tensor("ident", [B, B], f32, kind="ExternalInput")
    ones_d = nc.dram_tensor("ones", [1, B], f32r, kind="ExternalInput")
    idr_d = nc.dram_tensor("identr", [B, B], f32r, kind="ExternalInput")

    norm_d = nc.dram_tensor("norm_sh", [B, t_steps, HC], f32, kind="ExternalOutput")
    hout_d = nc.dram_tensor("h_sh", [B, HC], f32, kind="ExternalOutput")
    cout_d = nc.dram_tensor("c_sh", [B, HC], f32, kind="ExternalOutput")

    # ---- internal DRAM: per-step AllGather outputs (= y0 history for layer 1)
    y0hist = nc.dram_tensor("y0hist", [t_steps, H, B], f32, addr_space="Shared")
    y1hist = nc.dram_tensor("y1hist", [t_steps, H, B], f32, addr_space="Shared")

    rg = [list(range(NCORES))]

    with tile.TileContext(nc) as tc:
        with (
            tc.tile_pool(name="wpool", bufs=1) as wpool,
            tc.tile_pool(name="xts", bufs=1) as xtspool,
            tc.tile_pool(name="gath", bufs=6) as gath,
            tc.tile_pool(name="gates", bufs=4) as gp,
            tc.tile_pool(name="cstate", bufs=2) as cpool,
            tc.tile_pool(name="hstate", bufs=3) as hpool,
            tc.tile_pool(name="htall", bufs=3) as htall,
            tc.tile_pool(name="psz", bufs=3, space="PSUM") as psz,
            tc.tile_pool(name="ps1", bufs=2, space="PSUM") as ps1,
            tc.tile_pool(name="pstr", bufs=2, space="PSUM") as pstr,
            tc.tile_pool(name="dram", bufs=6, space="DRAM") as dpool,
        ):
            # ---- static SBUF loads ----
            u0s = wpool.tile([128, KH, G], f32)
            nc.sync.dma_start(u0s[:], u0_d.rearrange("k p n -> p k n"))
            u1s = wpool.tile([128, KH, G], f32)
            nc.sync.dma_start(u1s[:], u1_d.rearrange("k p n -> p k n"))
            w1s = wpool.tile([128, KH, G], f32)
            nc.sync.dma_start(w1s[:], w1_d.rearrange("k p n -> p k n"))
            w0s = wpool.tile([128, KD, G], f32)
            nc.sync.dma_start(w0s[:], w0_d.rearrange("k p n -> p k n"))
            b0s = wpool.tile([1, G], f32)
            nc.sync.dma_start(b0s[:], b0_d[:])
            b1s = wpool.tile([1, G], f32)
            nc.sync.dma_start(b1s[:], b1_d[:])
            toks = wpool.tile([B, t_steps], i32)
            nc.sync.dma_start(toks[:], tok_d[:])
            sbs = wpool.tile([B, HC], f32)
            nc.sync.dma_start(sbs[:], sb_d[:])
            tbs = wpool.tile([B, HC], f32)
            nc.sync.dma_start(tbs[:], tb_d[:])
            ident = wpool.tile([B, B], f32)
            nc.sync.dma_start(ident[:], id_d[:])
            ones = wpool.tile([1, B], f32)
            nc.gpsimd.memset(ones[:], 1.0)
            h0ts = wpool.tile([128, KH, B], f32)
            nc.sync.dma_start(h0ts[:], h0t_d.rearrange("(k p) b -> p k b", p=128))
            c0s = cpool.tile([B, HC], f32)
            nc.sync.dma_start(c0s[:], c0_d[:])

            # x^T tiles for all steps: [128, t, KD, B]
            xts = xtspool.tile([128, t_steps, KD, B], f32)

            def fetch_x(t):
                xg = gath.tile([B, D], f32, tag="xg")
                nc.gpsimd.indirect_dma_start(
                    out=xg[:],
                    out_offset=None,
                    in_=emb_d[:],
                    in_offset=bass.IndirectOffsetOnAxis(ap=toks[:, t : t + 1], axis=0),
                )
                for d in range(KD):
                    ps = pstr.tile([128, B], f32, tag="ptr")
                    nc.tensor.transpose(ps[:], xg[:, d * 128 : (d + 1) * 128], ident[:])
                    nc.vector.tensor_copy(xts[:, t, d, :], ps[:])

            LOOKAHEAD = 3
            for t in range(min(LOOKAHEAD, t_steps)):
                fetch_x(t)

            def lstm_step(zps, hT, c_prev):
                """Gate math from accumulated z psum [B, G]; returns (h_new, c_new)
                and fills hT [128, B] (SBUF, transposed h) if hT is not None."""
                sif = gp.tile([B, 2 * HC], f32, tag="sif")
                nc.scalar.activation(sif[:], zps[:, 0 : 2 * HC], AF.Sigmoid)
                tg = gp.tile([B, HC], f32, tag="tg")
                nc.scalar.activation(tg[:], zps[:, 2 * HC : 3 * HC], AF.Tanh)
                so = gp.tile([B, HC], f32, tag="so")
                nc.scalar.activation(so[:], zps[:, 3 * HC : 4 * HC], AF.Sigmoid)
                t1 = gp.tile([B, HC], f32, tag="t1")
                nc.vector.tensor_mul(t1[:], sif[:, HC : 2 * HC], c_prev[:])
                t2 = gp.tile([B, HC], f32, tag="t2")
                nc.vector.tensor_mul(t2[:], sif[:, 0:HC], tg[:])
                c_new = cpool.tile([B, HC], f32, tag="c")
                nc.vector.tensor_add(c_new[:], t1[:], t2[:])
                tcn = gp.tile([B, HC], f32, tag="tc")
                nc.scalar.activation(tcn[:], c_new[:], AF.Tanh)
                h_new = hpool.tile([B, HC], f32, tag="h")
                nc.vector.tensor_mul(h_new[:], so[:], tcn[:])
                return h_new, c_new

            def transpose_h(h_new):
                ps = pstr.tile([128, B], f32, tag="ptr")
                nc.tensor.transpose(ps[:], h_new[:], ident[:])
                hT = hpool.tile([128, B], f32, tag="hT")
                nc.vector.tensor_copy(hT[:], ps[:])
                return hT

            def allgather_h(hT, hist, t):
                agin = dpool.tile([128, B], f32, tag="agin")
                nc.sync.dma_start(agin[:], hT[:])
                nc.gpsimd.collective_compute(
                    "AllGather",
                    mybir.AluOpType.bypass,
                    replica_groups=rg,
                    ins=[agin[:].opt()],
                    outs=[hist[t].opt()],
                )
                hTall = htall.tile([128, KH, B], f32, tag="hTall")
                nc.sync.dma_start(
                    hTall[:], hist[t].rearrange("(k p) b -> p k b", p=128)
                )
                return hTall

            # =================== layer 0 ===================
            hT_cur = h0ts
            c_prev = c0s
            for t in range(t_steps):
                zps = psz.tile([B, G], f32, tag="z")
                # x @ W0 part first (independent of the recurrence wait)
                for d in range(KD):
                    nc.tensor.matmul(
                        zps[:],
                        xts[:, t, d, :].bitcast(f32r),
                        w0s[:, d, :].bitcast(f32r),
                        start=(d == 0),
                        stop=False,
                    )
                nc.tensor.matmul(
                    zps[:], ones[:].bitcast(f32r), b0s[:].bitcast(f32r),
                    start=False, stop=False,
                )
                for k in range(KH):
                    nc.tensor.matmul(
                        zps[:],
                        hT_cur[:, k, :].bitcast(f32r),
                        u0s[:, k, :].bitcast(f32r),
                        start=False,
                        stop=(k == KH - 1),
                    )
                if t + LOOKAHEAD < t_steps:
                    fetch_x(t + LOOKAHEAD)
                h_new, c_new = lstm_step(zps, None, c_prev)
                hT = transpose_h(h_new)
                hT_cur = allgather_h(hT, y0hist, t)
                c_prev = c_new

            # =================== layer 1 ===================
            # initial state = final state of layer 0
            h1T_cur = hT_cur
            c_prev = c_new
            for t in range(t_steps):
                y0T = htall.tile([128, KH, B], f32, tag="y0T")
                nc.sync.dma_start(
                    y0T[:], y0hist[t].rearrange("(k p) b -> p k b", p=128)
                )
                zps = psz.tile([B, G], f32, tag="z")
                for k in range(KH):
                    nc.tensor.matmul(
                        zps[:],
                        y0T[:, k, :].bitcast(f32r),
                        w1s[:, k, :].bitcast(f32r),
                        start=(k == 0),
                        stop=False,
                    )
                nc.tensor.matmul(
                    zps[:], ones[:].bitcast(f32r), b1s[:].bitcast(f32r),
                    start=False, stop=False,
                )
                for k in range(KH):
                    nc.tensor.matmul(
                        zps[:],
                        h1T_cur[:, k, :].bitcast(f32r),
                        u1s[:, k, :].bitcast(f32r),
                        start=False,
                        stop=(k == KH - 1),
                    )
                h_new, c_new = lstm_step(zps, None, c_prev)
                # BN + output write (off the critical path)
                tmp = gp.tile([B, HC], f32, tag="bn1")
                nc.vector.tensor_mul(tmp[:], h_new[:], sbs[:])
                nrm = gp.tile([B, HC], f32, tag="bn2")
                nc.vector.tensor_add(nrm[:], tmp[:], tbs[:])
                nc.sync.dma_start(norm_d[:, t, :], nrm[:])
                if t < t_steps - 1:
                    hT = transpose_h(h_new)
                    h1T_cur = allgather_h(hT, y1hist, t)
                c_prev = c_new

            nc.sync.dma_start(hout_d[:], h_new[:])
            nc.sync.dma_start(cout_d[:], c_new[:])

    nc.compile()
    return nc


def _prep_inputs(inputs, t_steps=T):
    """Host-side shard prep. Returns in_maps (list of dicts, one per core)."""
    tokens = np.asarray(inputs["tokens"], dtype=np.int32)[:, :t_steps]
    emb = np.ascontiguousarray(np.asarray(inputs["emb"], dtype=np.float32))
    h0 = np.asarray(inputs["h0"], dtype=np.float32)
    c0 = np.asarray(inputs["c0"], dtype=np.float32)
    W0 = np.asarray(inputs["W0"], dtype=np.float32)
    U0 = np.asarray(inputs["U0"], dtype=np.float32)
    b0 = np.asarray(inputs["b0"], dtype=np.float32)
    W1 = np.asarray(inputs["W1"], dtype=np.float32)
    U1 = np.asarray(inputs["U1"], dtype=np.float32)
    b1 = np.asarray(inputs["b1"], dtype=np.float32)
    gamma = np.asarray(inputs["gamma"], dtype=np.float32)
    beta = np.asarray(inputs["beta"], dtype=np.float32)
    mov_mean = np.asarray(inputs["mov_mean"], dtype=np.float32)
    mov_var = np.asarray(inputs["mov_var"], dtype=np.float32)

    s = gamma / np.sqrt(mov_var + BN_EPS)
    tt = beta - mov_mean * s

    h0t = np.ascontiguousarray(h0.T)  # [H, B]
    ident = np.eye(B, dtype=np.float32)

    in_maps = []
    for k in range(NCORES):
        cols = _gate_cols(k)
        hsl = slice(k * HC, (k + 1) * HC)
        in_maps.append(
            {
                "emb": emb,
                "tok": np.ascontiguousarray(tokens),
                "u0": np.ascontiguousarray(U0[:, cols].reshape(KH, 128, G)),
                "u1": np.ascontiguousarray(U1[:, cols].reshape(KH, 128, G)),
                "w1": np.ascontiguousarray(W1[:, cols].reshape(KH, 128, G)),
                "w0": np.ascontiguousarray(W0[:, cols].reshape(KD, 128, G)),
                "b0": np.ascontiguousarray(b0[cols][None, :]),
                "b1": np.ascontiguousarray(b1[cols][None, :]),
                "h0t": h0t,
                "c0sh": np.ascontiguousarray(c0[:, hsl]),
                "sb": np.ascontiguousarray(
                    np.broadcast_to(s[hsl][None, :], (B, HC))
                ),
                "tb": np.ascontiguousarray(
                    np.broadcast_to(tt[hsl][None, :], (B, HC))
                ),
                "ident": ident,
                "ones": np.ones((1, B), dtype=np.float32),
                "identr": ident,
            }
        )
    return in_maps


def run(inputs, t_steps=T, trace=False):
    import concourse.bass_utils as bass_utils

    if t_steps not in _COMPILED:
        _COMPILED[t_steps] = build_program(t_steps)
    nc = _COMPILED[t_steps]
    in_maps = _prep_inputs(inputs, t_steps)
    res = bass_utils.run_bass_kernel_spmd(
        nc, in_maps, core_ids=list(range(NCORES)), trace=trace
    )
    results = res.results
    norm = np.concatenate([results[k]["norm_sh"] for k in range(NCORES)], axis=-1)
    h = np.concatenate([results[k]["h_sh"] for k in range(NCORES)], axis=-1)
    c = np.concatenate([results[k]["c_sh"] for k in range(NCORES)], axis=-1)
    return (norm, h, c), res


def kernel(**inputs):
    (norm, h, c), _ = run(inputs, t_steps=T)
    return norm, h, c


# revision 14
# speedup vs baseline: 1.2230x; 1.2230x over previous
"""Trainium2 Bass kernel for a 2-layer LSTM encoder (B=64, T=128, D=256, H=1024)
with embedding lookup and inference BatchNorm.

Sharding: model-parallel over the hidden dimension H (each of 8 cores owns a
128-wide H chunk = 512 of the 4096 gate columns). The recurrence is sequential
in T; after each step every core broadcasts its h chunk to all cores via an
AllGather so the next step's h @ U matmul has the full hidden state.

Matmul orientation: out z[t] = [B=64, 512_own_gates] with the small h^T / x^T
chunks as the stationary operand and the (SBUF-resident) weight chunks as the
moving operand at N=512 — float32r streams at full rate for N >= 256.
"""

import sys

import numpy as np

for _p in ("/opt/trn_rl_repo", "/root/.axon_site/_ro/trn_rl_repo"):
    if _p not in sys.path:
        sys.path.append(_p)

B = 64
T = 128
V = 32000
D = 256
H = 1024
NCORES = 8
HC = H // NCORES        # 128 hidden dims owned per core
G = 4 * HC              # 512 gate columns owned per core
KH = H // 128           # 8 K-chunks for H contraction
KD = D // 128           # 2 K-chunks for D contraction
BN_EPS = 1e-3

_COMPILED = {}


def _gate_cols(k):
    """Columns of the 4H gate dim owned by core k, permuted to [i, f, o, g]
    so sigmoid covers one contiguous span (Keras stores i,f,g,o)."""
    return np.concatenate(
        [np.arange(g * H + k * HC, g * H + (k + 1) * HC) for g in (0, 1, 3, 2)]
    )


def build_program(t_steps=T):
    import concourse.bass as bass
    import concourse.bacc as bacc
    import concourse.mybir as mybir
    import concourse.tile as tile
    from concourse._compat import axon_active

    f32 = mybir.dt.float32
    f32r = mybir.dt.float32r
    i32 = mybir.dt.int32
    AF = mybir.ActivationFunctionType

    nc = bacc.Bacc(
        "TRN2",
        target_bir_lowering=False,
        debug=not axon_active(),
        enable_asserts=True,
        num_devices=NCORES,
    )

    # ---- DRAM I/O (per-core shards supplied via in_maps) ----
    emb_d = nc.dram_tensor("emb", [V, D], f32, kind="ExternalInput")
    tok_d = nc.dram_tensor("tok", [128, t_steps // 2], i32, kind="ExternalInput")
    u0_d = nc.dram_tensor("u0", [KH, 128, G], f32, kind="ExternalInput")
    u1_d = nc.dram_tensor("u1", [KH, 128, G], f32, kind="ExternalInput")
    w1_d = nc.dram_tensor("w1", [KH, 128, G], f32, kind="ExternalInput")
    w0_d = nc.dram_tensor("w0", [KD, 128, G], f32, kind="ExternalInput")
    b0_d = nc.dram_tensor("b0", [1, G], f32, kind="ExternalInput")
    b1_d = nc.dram_tensor("b1", [1, G], f32, kind="ExternalInput")
    h0t_d = nc.dram_tensor("h0t", [H, B], f32, kind="ExternalInput")
    c0_d = nc.dram_tensor("c0sh", [B, HC], f32, kind="ExternalInput")
    sb_d = nc.dram_tensor("sb", [B, HC], f32, kind="ExternalInput")
    tb_d = nc.dram_tensor("tb", [B, HC], f32, kind="ExternalInput")
    id_d = nc.dram_tensor("ident", [128, B], f32, kind="ExternalInput")
    ones_d = nc.dram_tensor("ones", [1, B], f32r, kind="ExternalInput")
    idr_d = nc.dram_tensor("identr", [B, B], f32r, kind="ExternalInput")

    norm_d = nc.dram_tensor("norm_sh", [B, t_steps, HC], f32, kind="ExternalOutput")
    hout_d = nc.dram_tensor("h_sh", [B, HC], f32, kind="ExternalOutput")
    cout_d = nc.dram_tensor("c_sh", [B, HC], f32, kind="ExternalOutput")

    # ---- internal DRAM: per-step AllGather outputs (= y0 history for layer 1)
    y0hist = nc.dram_tensor("y0hist", [t_steps, H, B], f32, addr_space="Shared")
    y1hist = nc.dram_tensor("y1hist", [t_steps, H, B], f32, addr_space="Shared")

    rg = [list(range(NCORES))]

    with tile.TileContext(nc) as tc:
        with (
            tc.tile_pool(name="wpool", bufs=1) as wpool,
            tc.tile_pool(name="xts", bufs=1) as xtspool,
            tc.tile_pool(name="gath", bufs=6) as gath,
            tc.tile_pool(name="gates", bufs=6) as gp,
            tc.tile_pool(name="cstate", bufs=2) as cpool,
            tc.tile_pool(name="hstate", bufs=4) as hpool,
            tc.tile_pool(name="htall", bufs=4) as htall,
            tc.tile_pool(name="psz", bufs=4, space="PSUM") as psz,
            tc.tile_pool(name="ps1", bufs=2, space="PSUM") as ps1,
            tc.tile_pool(name="pstr", bufs=2, space="PSUM") as pstr,
            tc.tile_pool(name="dram", bufs=8, space="DRAM") as dpool,
        ):
            # ---- static SBUF loads ----
            u0s = wpool.tile([128, KH, G], f32)
            nc.sync.dma_start(u0s[:], u0_d.rearrange("k p n -> p k n"))
            u1s = wpool.tile([128, KH, G], f32)
            nc.sync.dma_start(u1s[:], u1_d.rearrange("k p n -> p k n"))
            w1s = wpool.tile([128, KH, G], f32)
            nc.sync.dma_start(w1s[:], w1_d.rearrange("k p n -> p k n"))
            w0s = wpool.tile([128, KD, G], f32)
            nc.sync.dma_start(w0s[:], w0_d.rearrange("k p n -> p k n"))
            b0s = wpool.tile([1, G], f32)
            nc.sync.dma_start(b0s[:], b0_d[:])
            b1s = wpool.tile([1, G], f32)
            nc.sync.dma_start(b1s[:], b1_d[:])
            toks = wpool.tile([128, t_steps // 2], i32)
            nc.sync.dma_start(toks[:], tok_d[:])
            sbs = wpool.tile([B, HC], f32)
            nc.sync.dma_start(sbs[:], sb_d[:])
            tbs = wpool.tile([B, HC], f32)
            nc.sync.dma_start(tbs[:], tb_d[:])
            ident = wpool.tile([128, B], f32)
            nc.sync.dma_start(ident[:], id_d[:])
            ones = wpool.tile([1, B], f32)
            nc.gpsimd.memset(ones[:], 1.0)
            h0ts = wpool.tile([128, KH, B], f32)
            nc.sync.dma_start(h0ts[:], h0t_d.rearrange("(k p) b -> p k b", p=128))
            c0s = cpool.tile([B, HC], f32)
            nc.sync.dma_start(c0s[:], c0_d[:])

            # x^T tiles for all steps: [128, t, KD, B]
            xts = xtspool.tile([128, t_steps, KD, B], f32)

            xg_pairs = {}

            def fetch_x(t):
                # gather TWO steps per indirect DMA: 128 rows, one per partition
                j = t // 2
                if j not in xg_pairs:
                    xg = gath.tile([128, D], f32, tag="xg", name=f"xg{j}")
                    nc.gpsimd.indirect_dma_start(
                        out=xg[:],
                        out_offset=None,
                        in_=emb_d[:],
                        in_offset=bass.IndirectOffsetOnAxis(
                            ap=toks[:, j : j + 1], axis=0
                        ),
                    )
                    xg_pairs[j] = xg
                xg = xg_pairs[j]
                half = (t % 2) * B
                for d in range(KD):
                    ps = pstr.tile([128, B], f32, tag="ptr")
                    nc.tensor.transpose(
                        ps[:],
                        xg[half : half + B, d * 128 : (d + 1) * 128],
                        ident[half : half + B, :],
                    )
                    nc.vector.tensor_copy(xts[:, t, d, :], ps[:])

            LOOKAHEAD = 8
            for t in range(min(LOOKAHEAD, t_steps)):
                fetch_x(t)

            def lstm_step(zps, hT, c_prev):
                """Gate math from accumulated z psum [B, G]; returns (h_new, c_new)
                and fills hT [128, B] (SBUF, transposed h) if hT is not None."""
                # gate columns are host-permuted to [i, f, o, g]
                sif = gp.tile([B, 3 * HC], f32, tag="sif")
                nc.scalar.activation(sif[:], zps[:, 0 : 3 * HC], AF.Sigmoid)
                tg = gp.tile([B, HC], f32, tag="tg")
                nc.scalar.activation(tg[:], zps[:, 3 * HC : 4 * HC], AF.Tanh)
                so = sif[:, 2 * HC : 3 * HC]
                t1 = gp.tile([B, HC], f32, tag="t1")
                nc.vector.tensor_mul(t1[:], sif[:, HC : 2 * HC], c_prev[:])
                t2 = gp.tile([B, HC], f32, tag="t2")
                nc.vector.tensor_mul(t2[:], sif[:, 0:HC], tg[:])
                c_new = cpool.tile([B, HC], f32, tag="c")
                nc.vector.tensor_add(c_new[:], t1[:], t2[:])
                tcn = gp.tile([B, HC], f32, tag="tc")
                nc.scalar.activation(tcn[:], c_new[:], AF.Tanh)
                h_new = hpool.tile([B, HC], f32, tag="h")
                nc.vector.tensor_mul(h_new[:], so, tcn[:])
                return h_new, c_new

            def transpose_h(h_new):
                ps = pstr.tile([128, B], f32, tag="ptr")
                nc.tensor.transpose(ps[:], h_new[:], ident[0:B, :])
                hT = hpool.tile([128, B], f32, tag="hT")
                nc.vector.tensor_copy(hT[:], ps[:])
                return hT

            def allgather_h(hT, hist, t):
                agin = dpool.tile([128, B], f32, tag="agin")
                nc.sync.dma_start(agin[:], hT[:])
                nc.gpsimd.collective_compute(
                    "AllGather",
                    mybir.AluOpType.bypass,
                    replica_groups=rg,
                    ins=[agin[:].opt()],
                    outs=[hist[t].opt()],
                )
                hTall = htall.tile([128, KH, B], f32, tag="hTall")
                nc.sync.dma_start(
                    hTall[:], hist[t].rearrange("(k p) b -> p k b", p=128)
                )
                return hTall

            # =================== layer 0 ===================
            hT_cur = h0ts
            c_prev = c0s
            for t in range(t_steps):
                zps = psz.tile([B, G], f32, tag="z")
                # x @ W0 part first (independent of the recurrence wait)
                for d in range(KD):
                    nc.tensor.matmul(
                        zps[:],
                        xts[:, t, d, :].bitcast(f32r),
                        w0s[:, d, :].bitcast(f32r),
                        start=(d == 0),
                        stop=False,
                    )
                nc.tensor.matmul(
                    zps[:], ones[:].bitcast(f32r), b0s[:].bitcast(f32r),
                    start=False, stop=False,
                )
                for k in range(KH):
                    nc.tensor.matmul(
                        zps[:],
                        hT_cur[:, k, :].bitcast(f32r),
                        u0s[:, k, :].bitcast(f32r),
                        start=False,
                        stop=(k == KH - 1),
                    )
                if t + LOOKAHEAD < t_steps:
                    fetch_x(t + LOOKAHEAD)
                h_new, c_new = lstm_step(zps, None, c_prev)
                hT = transpose_h(h_new)
                hT_cur = allgather_h(hT, y0hist, t)
                c_prev = c_new

            # =================== layer 1 ===================
            # initial state = final state of layer 0
            h1T_cur = hT_cur
            c_prev = c_new
            for t in range(t_steps):
                y0T = htall.tile([128, KH, B], f32, tag="y0T")
                nc.sync.dma_start(
                    y0T[:], y0hist[t].rearrange("(k p) b -> p k b", p=128)
                )
                zps = psz.tile([B, G], f32, tag="z")
                for k in range(KH):
                    nc.tensor.matmul(
                        zps[:],
                        y0T[:, k, :].bitcast(f32r),
                        w1s[:, k, :].bitcast(f32r),
                        start=(k == 0),
                        stop=False,
                    )
                nc.tensor.matmul(
                    zps[:], ones[:].bitcast(f32r), b1s[:].bitcast(f32r),
                    start=False, stop=False,
                )
                for k in range(KH):
                    nc.tensor.matmul(
                        zps[:],
                        h1T_cur[:, k, :].bitcast(f32r),
                        u1s[:, k, :].bitcast(f32r),
                        start=False,
                        stop=(k == KH - 1),
                    )
                h_new, c_new = lstm_step(zps, None, c_prev)
                # BN + output write (off the critical path)
                tmp = gp.tile([B, HC], f32, tag="bn1")
                nc.vector.tensor_mul(tmp[:], h_new[:], sbs[:])
                nrm = gp.tile([B, HC], f32, tag="bn2")
                nc.vector.tensor_add(nrm[:], tmp[:], tbs[:])
                nc.sync.dma_start(norm_d[:, t, :], nrm[:])
                if t < t_steps - 1:
                    hT = transpose_h(h_new)
                    h1T_cur = allgather_h(hT, y1hist, t)
                c_prev = c_new

            nc.sync.dma_start(hout_d[:], h_new[:])
            nc.sync.dma_start(cout_d[:], c_new[:])

    nc.compile()
    return nc


def _prep_inputs(inputs, t_steps=T):
    """Host-side shard prep. Returns in_maps (list of dicts, one per core)."""
    tokens = np.asarray(inputs["tokens"], dtype=np.int32)[:, :t_steps]
    emb = np.ascontiguousarray(np.asarray(inputs["emb"], dtype=np.float32))
    h0 = np.asarray(inputs["h0"], dtype=np.float32)
    c0 = np.asarray(inputs["c0"], dtype=np.float32)
    W0 = np.asarray(inputs["W0"], dtype=np.float32)
    U0 = np.asarray(inputs["U0"], dtype=np.float32)
    b0 = np.asarray(inputs["b0"], dtype=np.float32)
    W1 = np.asarray(inputs["W1"], dtype=np.float32)
    U1 = np.asarray(inputs["U1"], dtype=np.float32)
    b1 = np.asarray(inputs["b1"], dtype=np.float32)
    gamma = np.asarray(inputs["gamma"], dtype=np.float32)
    beta = np.asarray(inputs["beta"], dtype=np.float32)
    mov_mean = np.asarray(inputs["mov_mean"], dtype=np.float32)
    mov_var = np.asarray(inputs["mov_var"], dtype=np.float32)

    # token pairs: gather row p of pair j = tokens[p % 64, 2j + p // 64]
    tok2 = np.empty((128, t_steps // 2), dtype=np.int32)
    tok2[:B, :] = tokens[:, 0::2]
    tok2[B:, :] = tokens[:, 1::2]

    s = gamma / np.sqrt(mov_var + BN_EPS)
    tt = beta - mov_mean * s

    h0t = np.ascontiguousarray(h0.T)  # [H, B]
    ident = np.vstack([np.eye(B, dtype=np.float32)] * 2)

    in_maps = []
    for k in range(NCORES):
        cols = _gate_cols(k)
        hsl = slice(k * HC, (k + 1) * HC)
        in_maps.append(
            {
                "emb": emb,
                "tok": tok2,
                "u0": np.ascontiguousarray(U0[:, cols].reshape(KH, 128, G)),
                "u1": np.ascontiguousarray(U1[:, cols].reshape(KH, 128, G)),
                "w1": np.ascontiguousarray(W1[:, cols].reshape(KH, 128, G)),
                "w0": np.ascontiguousarray(W0[:, cols].reshape(KD, 128, G)),
                "b0": np.ascontiguousarray(b0[cols][None, :]),
                "b1": np.ascontiguousarray(b1[cols][None, :]),
                "h0t": h0t,
                "c0sh": np.ascontiguousarray(c0[:, hsl]),
                "sb": np.ascontiguousarray(
                    np.broadcast_to(s[hsl][None, :], (B, HC))
                ),
                "tb": np.ascontiguousarray(
                    np.broadcast_to(tt[hsl][None, :], (B, HC))
                ),
                "ident": ident,
                "ones": np.ones((1, B), dtype=np.float32),
                "identr": np.eye(B, dtype=np.float32),
            }
        )
    return in_maps


def run(inputs, t_steps=T, trace=False):
    import concourse.bass_utils as bass_utils

    if t_steps not in _COMPILED:
        _COMPILED[t_steps] = build_program(t_steps)
    nc = _COMPILED[t_steps]
    in_maps = _prep_inputs(inputs, t_steps)
    res = bass_utils.run_bass_kernel_spmd(
        nc, in_maps, core_ids=list(range(NCORES)), trace=trace
    )
    results = res.results
    norm = np.concatenate([results[k]["norm_sh"] for k in range(NCORES)], axis=-1)
    h = np.concatenate([results[k]["h_sh"] for k in range(NCORES)], axis=-1)
    c = np.concatenate([results[k]["c_sh"] for k in range(NCORES)], axis=-1)
    return (norm, h, c), res


def kernel(**inputs):
    (norm, h, c), _ = run(inputs, t_steps=T)
    return norm, h, c
